# revision 7
# baseline (speedup 1.0000x reference)
"""Chi2 loss over ragged windows — Trainium2 Bass kernel.

Math (per sample b of B=4096, rows of length L=4096):
    len  = e_in - s_in            (in [1024, 3072])
    chi2 = sum_{j<len} ivar[b, s_in+j] * (flu[b, s_in+j] - out[b, s_out+j])^2
    result = mean_b(chi2 / len)

Strategy: pure data-parallel over the batch, 512 samples per core on 8
cores.  Samples are globally sorted by window length (descending) and
dealt round-robin to cores, so every core sees an identical length
profile and the single SPMD program's tile widths are tight for all
cores simultaneously.

Precision staging (tolerance is 2e-2 relative): all three arrays are
staged in fp8 (e3m4, ~1.5% rms quantization).  flu and ivar share the
same window offsets, so they are interleaved element-wise into one
array and fetched with a single indirect-DMA descriptor per sample; the
ivar lanes are zeroed outside each sample's valid window (and the rows
are zero-padded past L), so the ragged tail masks itself — no
iota/mask instructions.

Per 512-column chunk the compute is spread over three engines:
  PE :  d = I @ x + (-I) @ y      (two fp8 matmuls accumulating in PSUM)
  ACT:  d2 = Square(d)            (PSUM -> SBUF fp16)
  DVE:  acc = reduce(d2 * w)      (one fused TensorTensorReduce pass)
The host divides each sample's sum by its length and means (f64).

Tiles are fetched widest-first so the 994ns/gather SWDGE descriptor
generation stays hidden behind long transfers; the first tile's gathers
are split so compute starts early.
"""

import numpy as np
import ml_dtypes

import bass_rust
import concourse.bass as bass
import concourse.tile as tile
from concourse import mybir
from concourse.bass_utils import run_bass_kernel_spmd

B, L = 4096, 4096
N_CORES = 8
BPC = B // N_CORES          # samples per core
P = 128                     # SBUF partitions
TILES = BPC // P            # 128-sample tiles per core
MAX_W = 3072                # max window length
ILV_STRIDE = 2 * (L + MAX_W)  # interleaved x/w rows, zero-padded past 2L

f32 = mybir.dt.float32
f16 = mybir.dt.float16
f8 = mybir.dt.float8e3
i32 = mybir.dt.int32

NP_F8 = ml_dtypes.float8_e3m4
F8_MAX = 15.0

CHUNK = 1024                # compute chunk width
MM_MAX = 512                # max moving free-size per matmul
FIRST_SPLIT = 256           # head split of the first tile's gathers
PSUM_DT = mybir.dt.float32  # matmul requires f32 out ([P, 1024] = two banks)
PSUM_BUFS = 4
IO_BUFS = 5
SCR_BUFS = 6


def legalize_waits(nc):
    """This compiler build only accepts one sync wait per instruction; hoist
    extra waits into standalone single-wait EventSemaphore instructions."""
    n = 0
    for func in nc.m.functions:
        for blk in func.blocks:
            insts = blk.instructions
            out = []
            for inst in insts:
                si = inst.sync_info
                if si is not None and si.on_wait and len(si.on_wait) > 1:
                    waits = list(si.on_wait)
                    for w in waits[:-1]:
                        n += 1
                        out.append(
                            bass_rust.InstEventSemaphore(
                                name=f"splitwait_{n}_{inst.name}",
                                engine=inst.engine,
                                ins=[],
                                outs=[],
                                sync_info=mybir.SyncInfo(on_wait=[w], on_update=[]),
                            )
                        )
                    inst.sync_info = mybir.SyncInfo(
                        on_wait=[waits[-1]], on_update=list(si.on_update)
                    )
                out.append(inst)
            if len(out) != len(insts):
                blk.instructions[:] = out
    return n


def make_chunks(W, start=0):
    """Split [start, W) into balanced chunks of <= CHUNK columns."""
    span = W - start
    n = max(1, -(-span // CHUNK))
    base = span // n
    rem = span - base * n
    out = []
    lo = start
    for i in range(n):
        hi = lo + base + (1 if i < rem else 0)
        out.append((lo, hi))
        lo = hi
    return out


def make_work(widths):
    """Compute chunks (t, lo, hi, col), tile-major, wide tiles first.
    Tile 0's first FIRST_SPLIT columns form their own chunk (they arrive
    first via the split gathers)."""
    work = []
    col = 0
    for t in range(TILES):
        if t == 0 and 0 < FIRST_SPLIT < widths[t]:
            chunks = [(0, FIRST_SPLIT)] + make_chunks(widths[t], FIRST_SPLIT)
        else:
            chunks = make_chunks(widths[t])
        for lo, hi in chunks:
            work.append((t, lo, hi, col))
            col += 1
    return work, col


def build_bass(widths, scratch=32768):
    work, ncol = make_work(widths)

    nc = bass.Bass(dynamic_dma_scratch_size=scratch)

    ilv = nc.dram_tensor("ilv", [BPC, ILV_STRIDE], f8, kind="ExternalInput")
    ydat = nc.dram_tensor("ydat", [BPC + 1, L], f8, kind="ExternalInput")
    idx = nc.dram_tensor("idx", [P, 3 * TILES], i32, kind="ExternalInput")
    ident = nc.dram_tensor("ident", [P, 2 * P], f8, kind="ExternalInput")
    res = nc.dram_tensor("res", [P, ncol], f32, kind="ExternalOutput")

    with tile.TileContext(nc) as tc:
        with (
            tc.tile_pool(name="sc", bufs=1) as sc,
            tc.tile_pool(name="io", bufs=IO_BUFS) as io,
            tc.tile_pool(name="scr", bufs=SCR_BUFS) as scr,
            tc.psum_pool(name="ps", bufs=PSUM_BUFS) as ps,
        ):
            idx_sb = sc.tile([P, 3 * TILES], i32)
            id_sb = sc.tile([P, 2 * P], f8)
            acc = sc.tile([P, ncol], f32)

            nc.sync.dma_start(out=idx_sb[:], in_=idx[:])
            nc.sync.dma_start(out=id_sb[:], in_=ident[:])

            def gather(dram, c, width, elem_off, tag):
                ti = io.tile([P, width], f8, tag=tag)
                nc.gpsimd.indirect_dma_start(
                    out=ti[:], out_offset=None, in_=dram[:],
                    in_offset=bass.IndirectOffsetOnAxis(
                        ap=idx_sb[:, c : c + 1], axis=1
                    ),
                    element_offset=elem_off,
                )
                return ti

            # gathers: tile 0 (widest) is split so compute starts early
            g_ilv = {}
            g_y = {}
            for t in range(TILES):
                W = widths[t]
                if t == 0 and 0 < FIRST_SPLIT < W:
                    ya = gather(ydat, 1, FIRST_SPLIT, 0, "ya")
                    ia = gather(ilv, 0, 2 * FIRST_SPLIT, 0, "ia")
                    yb = gather(ydat, 1, W - FIRST_SPLIT, FIRST_SPLIT, "y")
                    ib = gather(ilv, 0, 2 * (W - FIRST_SPLIT), 2 * FIRST_SPLIT, "i")
                    g_y[t] = (FIRST_SPLIT, ya, yb)
                    g_ilv[t] = (FIRST_SPLIT, ia, ib)
                else:
                    g_y[t] = (W, gather(ydat, 3 * t + 1, W, 0, "y"), None)
                    g_ilv[t] = (W, gather(ilv, 3 * t, 2 * W, 0, "i"), None)

            def slices(t, lo, hi):
                """(x_ap, w_ap, y_ap) for tile t columns [lo, hi)."""
                cut, ia, ib = g_ilv[t]
                cut_y, ya, yb = g_y[t]
                assert cut == cut_y
                if hi <= cut:
                    i_t, i_lo, i_hi = ia, lo, hi
                    y_ap = ya[:, lo:hi]
                else:
                    assert lo >= cut, "chunk straddles the split"
                    i_t, i_lo, i_hi = ib, lo - cut, hi - cut
                    y_ap = yb[:, lo - cut : hi - cut]
                x_ap = i_t[:, 2 * i_lo : 2 * i_hi : 2]
                w_ap = i_t[:, 2 * i_lo + 1 : 2 * i_hi : 2]
                return x_ap, w_ap, y_ap

            n = len(work)
            for k, (t, lo, hi, col) in enumerate(work):
                w = hi - lo
                d = ps.tile([P, w], PSUM_DT, tag="d")
                for mlo in range(0, w, MM_MAX):
                    mhi = min(w, mlo + MM_MAX)
                    x_ap, w_ap_, y_ap = slices(t, lo + mlo, lo + mhi)
                    nc.tensor.matmul(
                        d[:, mlo:mhi], id_sb[:, 0:P], x_ap, start=True, stop=False
                    )
                    nc.tensor.matmul(
                        d[:, mlo:mhi], id_sb[:, P : 2 * P], y_ap,
                        start=False, stop=True,
                    )
                _, w_ap, _ = slices(t, lo, hi)
                d2 = scr.tile([P, w], f16, tag="d2")
                if k == n - 1:
                    # drain chunk: keep the tail on one engine (DVE)
                    nc.vector.tensor_tensor(
                        out=d2[:], in0=d[:], in1=d[:], op=mybir.AluOpType.mult
                    )
                else:
                    nc.scalar.activation(
                        out=d2[:], in_=d[:],
                        func=mybir.ActivationFunctionType.Square,
                    )
                tt = scr.tile([P, w], f16, tag="tt")
                nc.vector.tensor_tensor_reduce(
                    out=tt[:], in0=d2[:], in1=w_ap, scale=1.0, scalar=0.0,
                    op0=mybir.AluOpType.mult, op1=mybir.AluOpType.add,
                    accum_out=acc[:, col : col + 1],
                )

            nc.sync.dma_start(out=res[:], in_=acc[:])

    legalize_waits(nc)
    return nc, work


def prepare_inputs(fluctuate, ivar, output, overlap_index):
    """Globally sort samples by window length, deal round-robin to cores,
    stage fp8 interleaved x/w (window-masked) and fp8 y per core."""
    flu = np.ascontiguousarray(fluctuate.reshape(B, L), dtype=np.float32)
    ivr = np.ascontiguousarray(ivar.reshape(B, L), dtype=np.float32)
    oup = np.ascontiguousarray(output.reshape(B, L), dtype=np.float32)
    oi = np.asarray(overlap_index)
    s_in = oi[:, 0].astype(np.int64)
    e_in = oi[:, 1].astype(np.int64)
    s_out = oi[:, 2].astype(np.int64)
    all_lens = e_in - s_in

    order = np.argsort(-all_lens, kind="stable")   # global, descending
    lens_sorted = all_lens[order]

    # tile t's width: the longest window among ranks [1024t, 1024(t+1))
    widths = []
    for t in range(TILES):
        w = int(lens_sorted[t * P * N_CORES])
        widths.append(min(MAX_W, -(-w // 32) * 32))

    x8 = NP_F8(np.clip(flu, -F8_MAX, F8_MAX))
    y8 = NP_F8(np.clip(oup, -F8_MAX, F8_MAX))
    w8 = NP_F8(ivr)

    ident = np.zeros((P, 2 * P), dtype=NP_F8)
    ident[:, :P] = NP_F8(np.eye(P, dtype=np.float32))
    ident[:, P:] = NP_F8(-np.eye(P, dtype=np.float32))

    jj = np.arange(L)
    in_maps = []
    core_lens = []
    for c in range(N_CORES):
        g = order[c::N_CORES]                      # this core's samples, sorted
        core_lens.append(all_lens[g].reshape(TILES, P))

        win = (jj[None, :] >= s_in[g, None]) & (jj[None, :] < e_in[g, None])
        ilv = np.zeros((BPC, ILV_STRIDE), dtype=NP_F8)
        ilv[:, 0 : 2 * L : 2] = x8[g]
        ilv[:, 1 : 2 * L : 2] = np.where(win, w8[g], NP_F8(0))
        ydat = np.zeros((BPC + 1, L), dtype=NP_F8)
        ydat[:BPC] = y8[g]

        rows = np.arange(BPC)
        idxm = np.empty((P, 3 * TILES), dtype=np.int32)
        for t in range(TILES):
            sl = slice(t * P, (t + 1) * P)
            idxm[:, 3 * t + 0] = rows[sl] * ILV_STRIDE + 2 * s_in[g][sl]
            idxm[:, 3 * t + 1] = rows[sl] * L + s_out[g][sl]
            idxm[:, 3 * t + 2] = 0

        in_maps.append(
            {"ilv": ilv, "ydat": ydat, "idx": idxm, "ident": ident}
        )

    return in_maps, widths, core_lens


def finish(results, work, core_lens):
    """Combine per-core per-chunk partial sums into the scalar mean."""
    total = 0.0
    for c in range(N_CORES):
        res = results[c]["res"].astype(np.float64)     # [P, ncol]
        sums = np.zeros((TILES, P), dtype=np.float64)
        for (t, lo, hi, col) in work:
            sums[t] += res[:, col]
        lens = core_lens[c].astype(np.float64)
        total += float((sums / lens).sum())
    return np.float32(total / B)


def kernel(fluctuate, ivar, output, overlap_index, _trace=False, **_kw):
    in_maps, widths, core_lens = prepare_inputs(
        fluctuate, ivar, output, overlap_index
    )
    nc, work = build_bass(widths)
    out = run_bass_kernel_spmd(
        nc, in_maps, core_ids=list(range(N_CORES)), trace=_trace
    )
    result = finish(out.results, work, core_lens)
    if _trace:
        return result, out
    return result


# revision 20
# speedup vs baseline: 1.2565x; 1.2565x over previous
"""Chi2 loss over ragged windows — Trainium2 Bass kernel.

Math (per sample b of B=4096, rows of length L=4096):
    len  = e_in - s_in            (in [1024, 3072])
    chi2 = sum_{j<len} ivar[b, s_in+j] * (flu[b, s_in+j] - out[b, s_out+j])^2
    result = mean_b(chi2 / len)

Strategy: pure data-parallel over the batch, 512 samples per core on 8
cores.  Samples are globally sorted by window length (descending) and
dealt round-robin to cores, so every core sees an identical length
profile and the single SPMD program's tile widths are tight for all
cores simultaneously.

Precision staging (tolerance is 2e-2 relative): all three arrays are
staged in fp8 (e3m4, ~1.5% rms quantization).  flu and sqrt(ivar) share
the same window offsets, so they are interleaved element-wise into one
array and fetched with a single indirect-DMA descriptor per sample; the
sqrt(ivar) lanes are zeroed outside each sample's valid window (and the
rows are zero-padded past 2L), so the ragged tail masks itself — no
iota/mask instructions.

Per compute chunk the work is spread over three engines (sw = sqrt(ivar)
is staged so the weight multiply precedes the square, collapsing
square+weight+reduce into two single-pass ops):
  PE :  d = I @ x + (-I) @ y      (two fp8 matmuls accumulating in PSUM)
  DVE:  t = d * sw                (PSUM f32 x fp8 -> SBUF fp16)
  ACT:  acc[col] = sum Square(t)  (one activation pass with accum_out)
The host divides each sample's sum by its length and means (f64).

Tiles are fetched widest-first so the 994ns/gather SWDGE descriptor
generation stays hidden behind long transfers; the first tile's gathers
are split so compute starts early, and dummy identity matmuls keep the
PE p-state ramped before real data lands.
"""

import numpy as np
import ml_dtypes

import bass_rust
import concourse.bass as bass
import concourse.tile as tile
from concourse import mybir
from concourse.bass_utils import run_bass_kernel_spmd

B, L = 4096, 4096
N_CORES = 8
BPC = B // N_CORES          # samples per core
P = 128                     # SBUF partitions
TILES = BPC // P            # 128-sample tiles per core
MAX_W = 3072                # max window length
ILV_STRIDE = 2 * (L + MAX_W)  # interleaved x/w rows, zero-padded past 2L

f32 = mybir.dt.float32
f16 = mybir.dt.float16
f8 = mybir.dt.float8e3
i32 = mybir.dt.int32

NP_F8 = ml_dtypes.float8_e3m4
F8_MAX = 15.0

# Tunables (swept against the calibrated cost-model timeline).
CFG = dict(
    chunk=1024,             # max compute-chunk width
    mm_max=512,             # max moving free-size per matmul
    splits={0: [1024]},     # per-tile extra gather cut points
    tile_order=(2, 1, 0, 3),  # gather/compute order (medium tiles first)
    tail_taper=(),          # optional small final chunks of the last tile
    warmup=26,              # dummy 128-row matmuls to hold the PE p-state
    act_group=2,            # DVE chunks per ACT square+accum pass
    dve_tail=0,             # final chunks square on DVE instead of ACT
    split_res=True,         # DMA all-but-last accum columns early
    io_bufs=6,
    scr_bufs=6,
    psum_bufs=3,            # [P, chunk] f32 tiles; 2 banks each + warm bank
)


def legalize_waits(nc):
    """This compiler build only accepts one sync wait per instruction; hoist
    extra waits into standalone single-wait EventSemaphore instructions."""
    n = 0
    for func in nc.m.functions:
        for blk in func.blocks:
            insts = blk.instructions
            out = []
            for inst in insts:
                si = inst.sync_info
                if si is not None and si.on_wait and len(si.on_wait) > 1:
                    waits = list(si.on_wait)
                    for w in waits[:-1]:
                        n += 1
                        out.append(
                            bass_rust.InstEventSemaphore(
                                name=f"splitwait_{n}_{inst.name}",
                                engine=inst.engine,
                                ins=[],
                                outs=[],
                                sync_info=mybir.SyncInfo(on_wait=[w], on_update=[]),
                            )
                        )
                    inst.sync_info = mybir.SyncInfo(
                        on_wait=[waits[-1]], on_update=list(si.on_update)
                    )
                out.append(inst)
            if len(out) != len(insts):
                blk.instructions[:] = out
    return n


def segments(widths, cfg):
    """Per tile: list of (seg_lo, seg_hi) gather segments."""
    segs = {}
    for t in range(TILES):
        cuts = [c for c in cfg["splits"].get(t, []) if 0 < c < widths[t]]
        pts = [0] + sorted(set(cuts)) + [widths[t]]
        segs[t] = list(zip(pts[:-1], pts[1:]))
    return segs


def make_work(widths, cfg):
    """Compute chunks (t, lo, hi, col): tiles in cfg order, chunk boundaries
    aligned to gather segments, each segment split into balanced <=chunk
    pieces.  The very last tile's tail is tapered into small chunks so the
    final PE->ACT->DVE drain chain is short."""
    segs = segments(widths, cfg)
    order = list(cfg["tile_order"])
    work = []
    col = 0
    for oi, t in enumerate(order):
        last_tile = oi == len(order) - 1
        for si, (slo, shi) in enumerate(segs[t]):
            last_seg = last_tile and si == len(segs[t]) - 1
            taper = list(cfg["tail_taper"]) if last_seg else []
            shi_main = shi - sum(taper)
            if shi_main < slo:        # taper doesn't fit; skip it
                taper, shi_main = [], shi
            span = shi_main - slo
            n = max(1, -(-span // cfg["chunk"]))
            base, rem = span // n, span % n
            lo = slo
            for i in range(n):
                hi = lo + base + (1 if i < rem else 0)
                if hi > lo:
                    work.append((t, lo, hi, col))
                    col += 1
                lo = hi
            for tw in taper:
                hi = lo + tw
                work.append((t, lo, hi, col))
                col += 1
                lo = hi
    return work, col


def make_groups(work, cfg):
    """Group consecutive same-tile chunks for one shared ACT square+accum
    pass; the final dve_tail chunks stay singleton groups."""
    n = len(work)
    groups = []
    cur = []
    for k, (t, lo, hi, col) in enumerate(work):
        tail = k >= n - cfg["dve_tail"]
        if cur and (tail or cur[0][0] != t or len(cur) >= cfg["act_group"]):
            groups.append(cur)
            cur = []
        cur.append((t, lo, hi, tail))
        if tail:
            groups.append(cur)
            cur = []
    if cur:
        groups.append(cur)
    return groups


def build_bass(widths, cfg=None, scratch=32768):
    cfg = dict(CFG, **(cfg or {}))
    work0, _ = make_work(widths, cfg)
    groups = make_groups(work0, cfg)
    # one accum column per group; `work` (returned for finish()) is group-level
    work = []
    for gcol, g in enumerate(groups):
        t = g[0][0]
        work.append((t, g[0][1], g[-1][2], gcol))
    ncol = len(groups)
    segs = segments(widths, cfg)

    nc = bass.Bass(dynamic_dma_scratch_size=scratch)

    ilv = nc.dram_tensor("ilv", [BPC, ILV_STRIDE], f8, kind="ExternalInput")
    ydat = nc.dram_tensor("ydat", [BPC + 1, L], f8, kind="ExternalInput")
    idx = nc.dram_tensor("idx", [P, 2 * TILES], i32, kind="ExternalInput")
    ident = nc.dram_tensor("ident", [P, 2 * P], f8, kind="ExternalInput")
    res = nc.dram_tensor("res", [P, ncol], f32, kind="ExternalOutput")

    with tile.TileContext(nc) as tc:
        with (
            tc.tile_pool(name="sc", bufs=1) as sc,
            tc.tile_pool(name="io", bufs=cfg["io_bufs"]) as io,
            tc.tile_pool(name="scr", bufs=cfg["scr_bufs"]) as scr,
            tc.psum_pool(name="ps", bufs=cfg["psum_bufs"]) as ps,
            tc.psum_pool(name="pw", bufs=1) as pw,
        ):
            idx_sb = sc.tile([P, 2 * TILES], i32)
            id_sb = sc.tile([P, 2 * P], f8)
            acc = sc.tile([P, ncol], f32)

            nc.sync.dma_start(out=idx_sb[:], in_=idx[:])
            nc.sync.dma_start(out=id_sb[:], in_=ident[:])

            # PE p-state warmup: dummy matmuls as soon as the identity lands
            warm = pw.tile([P, P], f32)
            for _ in range(cfg["warmup"]):
                nc.tensor.matmul(
                    warm[:], id_sb[:, 0:P], id_sb[:, 0:P], start=True, stop=True
                )

            def gather(dram, c, width, elem_off, tag):
                ti = io.tile([P, width], f8, tag=tag)
                nc.gpsimd.indirect_dma_start(
                    out=ti[:], out_offset=None, in_=dram[:],
                    in_offset=bass.IndirectOffsetOnAxis(
                        ap=idx_sb[:, c : c + 1], axis=1
                    ),
                    element_offset=elem_off,
                )
                return ti

            # gathers: per tile (in order), per segment: y then interleaved x/w
            g = {}          # (t, seg_idx) -> (y_tile, ilv_tile)
            for t in cfg["tile_order"]:
                for si, (slo, shi) in enumerate(segs[t]):
                    if cfg.get("ilv_first"):
                        it = gather(
                            ilv, 2 * t, 2 * (shi - slo), 2 * slo, f"i{si}"
                        )
                        yt = gather(ydat, 2 * t + 1, shi - slo, slo, f"y{si}")
                    else:
                        yt = gather(ydat, 2 * t + 1, shi - slo, slo, f"y{si}")
                        it = gather(
                            ilv, 2 * t, 2 * (shi - slo), 2 * slo, f"i{si}"
                        )
                    g[(t, si)] = (slo, shi, yt, it)

            def slices(t, lo, hi):
                """(x_ap, w_ap, y_ap) for tile t columns [lo, hi)."""
                for si in range(len(segs[t])):
                    slo, shi, yt, it = g[(t, si)]
                    if lo >= slo and hi <= shi:
                        a, b = lo - slo, hi - slo
                        return (
                            it[:, 2 * a : 2 * b : 2],
                            it[:, 2 * a + 1 : 2 * b : 2],
                            yt[:, a:b],
                        )
                raise AssertionError("chunk straddles a gather segment")

            for gcol, grp in enumerate(groups):
                gw = sum(hi - lo for (_, lo, hi, _) in grp)
                tt = scr.tile([P, gw], f16, tag="tt")
                off = 0
                for (t, lo, hi, tail) in grp:
                    w = hi - lo
                    d = ps.tile([P, w], f32, tag="d")
                    for mlo in range(0, w, cfg["mm_max"]):
                        mhi = min(w, mlo + cfg["mm_max"])
                        x_ap, _, y_ap = slices(t, lo + mlo, lo + mhi)
                        nc.tensor.matmul(
                            d[:, mlo:mhi], id_sb[:, 0:P], x_ap,
                            start=True, stop=False,
                        )
                        nc.tensor.matmul(
                            d[:, mlo:mhi], id_sb[:, P : 2 * P], y_ap,
                            start=False, stop=True,
                        )
                    _, w_ap, _ = slices(t, lo, hi)
                    nc.vector.tensor_tensor(
                        out=tt[:, off : off + w], in0=d[:], in1=w_ap,
                        op=mybir.AluOpType.mult,
                    )
                    off += w
                if grp[0][3]:
                    # drain chunks: square+reduce on DVE, skip the ACT chase
                    sq = scr.tile([P, gw], f16, tag="sq")
                    nc.vector.tensor_tensor(
                        out=sq[:], in0=tt[:], in1=tt[:], op=mybir.AluOpType.mult
                    )
                    nc.vector.tensor_reduce(
                        out=acc[:, gcol : gcol + 1], in_=sq[:],
                        axis=mybir.AxisListType.X, op=mybir.AluOpType.add,
                    )
                else:
                    d2 = scr.tile([P, gw], f16, tag="d2")
                    nc.scalar.activation(
                        out=d2[:], in_=tt[:],
                        func=mybir.ActivationFunctionType.Square,
                        accum_out=acc[:, gcol : gcol + 1],
                    )
                if cfg["split_res"] and gcol == ncol - 2:
                    # bulk result columns leave while the last group drains
                    nc.sync.dma_start(
                        out=res[:, : ncol - 1], in_=acc[:, : ncol - 1]
                    )

            if cfg["split_res"]:
                nc.sync.dma_start(
                    out=res[:, ncol - 1 : ncol], in_=acc[:, ncol - 1 : ncol]
                )
            else:
                nc.sync.dma_start(out=res[:], in_=acc[:])

    legalize_waits(nc)
    return nc, work


def prepare_inputs(fluctuate, ivar, output, overlap_index):
    """Globally sort samples by window length, deal round-robin to cores,
    stage fp8 interleaved x/sqrt(ivar) (window-masked) and fp8 y per core."""
    flu = np.ascontiguousarray(fluctuate.reshape(B, L), dtype=np.float32)
    ivr = np.ascontiguousarray(ivar.reshape(B, L), dtype=np.float32)
    oup = np.ascontiguousarray(output.reshape(B, L), dtype=np.float32)
    oi = np.asarray(overlap_index)
    s_in = oi[:, 0].astype(np.int64)
    e_in = oi[:, 1].astype(np.int64)
    s_out = oi[:, 2].astype(np.int64)
    all_lens = e_in - s_in

    order = np.argsort(-all_lens, kind="stable")   # global, descending
    lens_sorted = all_lens[order]

    # tile t's width: the longest window among ranks [1024t, 1024(t+1))
    widths = []
    for t in range(TILES):
        w = int(lens_sorted[t * P * N_CORES])
        widths.append(min(MAX_W, -(-w // 32) * 32))

    x8 = NP_F8(np.clip(flu, -F8_MAX, F8_MAX))
    y8 = NP_F8(np.clip(oup, -F8_MAX, F8_MAX))
    sw8 = NP_F8(np.sqrt(ivr, dtype=np.float32))

    ident = np.zeros((P, 2 * P), dtype=NP_F8)
    ident[:, :P] = NP_F8(np.eye(P, dtype=np.float32))
    ident[:, P:] = NP_F8(-np.eye(P, dtype=np.float32))

    jj = np.arange(L)
    in_maps = []
    core_lens = []
    for c in range(N_CORES):
        g = order[c::N_CORES]                      # this core's samples, sorted
        core_lens.append(all_lens[g].reshape(TILES, P))

        win = (jj[None, :] >= s_in[g, None]) & (jj[None, :] < e_in[g, None])
        ilv = np.zeros((BPC, ILV_STRIDE), dtype=NP_F8)
        ilv[:, 0 : 2 * L : 2] = x8[g]
        ilv[:, 1 : 2 * L : 2] = np.where(win, sw8[g], NP_F8(0))
        ydat = np.zeros((BPC + 1, L), dtype=NP_F8)
        ydat[:BPC] = y8[g]

        rows = np.arange(BPC)
        idxm = np.empty((P, 2 * TILES), dtype=np.int32)
        for t in range(TILES):
            sl = slice(t * P, (t + 1) * P)
            idxm[:, 2 * t + 0] = rows[sl] * ILV_STRIDE + 2 * s_in[g][sl]
            idxm[:, 2 * t + 1] = rows[sl] * L + s_out[g][sl]

        in_maps.append(
            {"ilv": ilv, "ydat": ydat, "idx": idxm, "ident": ident}
        )

    return in_maps, widths, core_lens


def finish(results, work, core_lens):
    """Combine per-core per-chunk partial sums into the scalar mean."""
    total = 0.0
    for c in range(N_CORES):
        res = results[c]["res"].astype(np.float64)     # [P, ncol]
        sums = np.zeros((TILES, P), dtype=np.float64)
        for (t, lo, hi, col) in work:
            sums[t] += res[:, col]
        lens = core_lens[c].astype(np.float64)
        total += float((sums / lens).sum())
    return np.float32(total / B)


def kernel(fluctuate, ivar, output, overlap_index, _trace=False, **_kw):
    in_maps, widths, core_lens = prepare_inputs(
        fluctuate, ivar, output, overlap_index
    )
    nc, work = build_bass(widths)
    out = run_bass_kernel_spmd(
        nc, in_maps, core_ids=list(range(N_CORES)), trace=_trace
    )
    result = finish(out.results, work, core_lens)
    if _trace:
        return result, out
    return result


# revision 21
# speedup vs baseline: 1.2803x; 1.0190x over previous
"""Chi2 loss over ragged windows — Trainium2 Bass kernel.

Math (per sample b of B=4096, rows of length L=4096):
    len  = e_in - s_in            (in [1024, 3072])
    chi2 = sum_{j<len} ivar[b, s_in+j] * (flu[b, s_in+j] - out[b, s_out+j])^2
    result = mean_b(chi2 / len)

Strategy: pure data-parallel over the batch, 512 samples per core on 8
cores.  Samples are globally sorted by window length (descending) and
dealt round-robin to cores, so every core sees an identical length
profile and the single SPMD program's tile widths are tight for all
cores simultaneously.

Precision staging (tolerance is 2e-2 relative): all three arrays are
staged in fp8 (e3m4, ~1.5% rms quantization).  flu and sqrt(ivar) share
the same window offsets, so they are interleaved element-wise into one
array and fetched with a single indirect-DMA descriptor per sample; the
sqrt(ivar) lanes are zeroed outside each sample's valid window (and the
rows are zero-padded past 2L), so the ragged tail masks itself — no
iota/mask instructions.

Per compute chunk the work is spread over three engines (sw = sqrt(ivar)
is staged so the weight multiply precedes the square, collapsing
square+weight+reduce into two single-pass ops):
  PE :  d = I @ x + (-I) @ y      (two fp8 matmuls accumulating in PSUM)
  DVE:  t = d * sw                (PSUM f32 x fp8 -> SBUF fp16)
  ACT:  acc[col] = sum Square(t)  (one activation pass with accum_out)
The host divides each sample's sum by its length and means (f64).

Tiles are fetched widest-first so the 994ns/gather SWDGE descriptor
generation stays hidden behind long transfers; the first tile's gathers
are split so compute starts early, and dummy identity matmuls keep the
PE p-state ramped before real data lands.
"""

import numpy as np
import ml_dtypes

import bass_rust
import concourse.bass as bass
import concourse.tile as tile
from concourse import mybir
from concourse.bass_utils import run_bass_kernel_spmd

B, L = 4096, 4096
N_CORES = 8
BPC = B // N_CORES          # samples per core
P = 128                     # SBUF partitions
TILES = BPC // P            # 128-sample tiles per core
MAX_W = 3072                # max window length
ILV_STRIDE = 2 * (L + MAX_W)  # interleaved x/w rows, zero-padded past 2L

f32 = mybir.dt.float32
f16 = mybir.dt.float16
f8 = mybir.dt.float8e3
i32 = mybir.dt.int32

NP_F8 = ml_dtypes.float8_e3m4
F8_MAX = 15.0

# Tunables (swept against the calibrated cost-model timeline).
CFG = dict(
    chunk=1024,             # max compute-chunk width
    mm_max=512,             # max moving free-size per matmul
    splits={0: [1024]},     # per-tile extra gather cut points
    tile_order=(3, 1, 0, 2),  # gather/compute order (narrow tile first)
    tail_taper=(),          # optional small final chunks of the last tile
    warmup=26,              # dummy 128-row matmuls to hold the PE p-state
    act_group=1,            # DVE chunks per ACT square+accum pass
    dve_tail=0,             # final chunks square on DVE instead of ACT
    split_res=True,         # DMA all-but-last accum columns early
    io_bufs=6,
    scr_bufs=6,
    psum_bufs=3,            # [P, chunk] f32 tiles; 2 banks each + warm bank
)


def legalize_waits(nc):
    """This compiler build only accepts one sync wait per instruction; hoist
    extra waits into standalone single-wait EventSemaphore instructions."""
    n = 0
    for func in nc.m.functions:
        for blk in func.blocks:
            insts = blk.instructions
            out = []
            for inst in insts:
                si = inst.sync_info
                if si is not None and si.on_wait and len(si.on_wait) > 1:
                    waits = list(si.on_wait)
                    for w in waits[:-1]:
                        n += 1
                        out.append(
                            bass_rust.InstEventSemaphore(
                                name=f"splitwait_{n}_{inst.name}",
                                engine=inst.engine,
                                ins=[],
                                outs=[],
                                sync_info=mybir.SyncInfo(on_wait=[w], on_update=[]),
                            )
                        )
                    inst.sync_info = mybir.SyncInfo(
                        on_wait=[waits[-1]], on_update=list(si.on_update)
                    )
                out.append(inst)
            if len(out) != len(insts):
                blk.instructions[:] = out
    return n


def segments(widths, cfg):
    """Per tile: list of (seg_lo, seg_hi) gather segments."""
    segs = {}
    for t in range(TILES):
        cuts = [c for c in cfg["splits"].get(t, []) if 0 < c < widths[t]]
        pts = [0] + sorted(set(cuts)) + [widths[t]]
        segs[t] = list(zip(pts[:-1], pts[1:]))
    return segs


def make_work(widths, cfg):
    """Compute chunks (t, lo, hi, col): tiles in cfg order, chunk boundaries
    aligned to gather segments, each segment split into balanced <=chunk
    pieces.  The very last tile's tail is tapered into small chunks so the
    final PE->ACT->DVE drain chain is short."""
    segs = segments(widths, cfg)
    order = list(cfg["tile_order"])
    work = []
    col = 0
    for oi, t in enumerate(order):
        last_tile = oi == len(order) - 1
        for si, (slo, shi) in enumerate(segs[t]):
            last_seg = last_tile and si == len(segs[t]) - 1
            taper = list(cfg["tail_taper"]) if last_seg else []
            shi_main = shi - sum(taper)
            if shi_main < slo:        # taper doesn't fit; skip it
                taper, shi_main = [], shi
            span = shi_main - slo
            n = max(1, -(-span // cfg["chunk"]))
            base, rem = span // n, span % n
            lo = slo
            for i in range(n):
                hi = lo + base + (1 if i < rem else 0)
                if hi > lo:
                    work.append((t, lo, hi, col))
                    col += 1
                lo = hi
            for tw in taper:
                hi = lo + tw
                work.append((t, lo, hi, col))
                col += 1
                lo = hi
    return work, col


def make_groups(work, cfg):
    """Group consecutive same-tile chunks for one shared ACT square+accum
    pass; the final dve_tail chunks stay singleton groups."""
    n = len(work)
    groups = []
    cur = []
    for k, (t, lo, hi, col) in enumerate(work):
        tail = k >= n - cfg["dve_tail"]
        if cur and (tail or cur[0][0] != t or len(cur) >= cfg["act_group"]):
            groups.append(cur)
            cur = []
        cur.append((t, lo, hi, tail))
        if tail:
            groups.append(cur)
            cur = []
    if cur:
        groups.append(cur)
    return groups


def build_bass(widths, cfg=None, scratch=32768):
    cfg = dict(CFG, **(cfg or {}))
    work0, _ = make_work(widths, cfg)
    groups = make_groups(work0, cfg)
    # one accum column per group; `work` (returned for finish()) is group-level
    work = []
    for gcol, g in enumerate(groups):
        t = g[0][0]
        work.append((t, g[0][1], g[-1][2], gcol))
    ncol = len(groups)
    segs = segments(widths, cfg)

    nc = bass.Bass(dynamic_dma_scratch_size=scratch)

    ilv = nc.dram_tensor("ilv", [BPC, ILV_STRIDE], f8, kind="ExternalInput")
    ydat = nc.dram_tensor("ydat", [BPC + 1, L], f8, kind="ExternalInput")
    idx = nc.dram_tensor("idx", [P, 2 * TILES], i32, kind="ExternalInput")
    ident = nc.dram_tensor("ident", [P, 2 * P], f8, kind="ExternalInput")
    res = nc.dram_tensor("res", [P, ncol], f32, kind="ExternalOutput")

    with tile.TileContext(nc) as tc:
        with (
            tc.tile_pool(name="sc", bufs=1) as sc,
            tc.tile_pool(name="io", bufs=cfg["io_bufs"]) as io,
            tc.tile_pool(name="scr", bufs=cfg["scr_bufs"]) as scr,
            tc.psum_pool(name="ps", bufs=cfg["psum_bufs"]) as ps,
            tc.psum_pool(name="pw", bufs=1) as pw,
        ):
            idx_sb = sc.tile([P, 2 * TILES], i32)
            id_sb = sc.tile([P, 2 * P], f8)
            acc = sc.tile([P, ncol], f32)

            nc.sync.dma_start(out=idx_sb[:], in_=idx[:])
            nc.sync.dma_start(out=id_sb[:], in_=ident[:])

            # PE p-state warmup: dummy matmuls as soon as the identity lands
            warm = pw.tile([P, P], f32)
            for _ in range(cfg["warmup"]):
                nc.tensor.matmul(
                    warm[:], id_sb[:, 0:P], id_sb[:, 0:P], start=True, stop=True
                )

            def gather(dram, c, width, elem_off, tag):
                ti = io.tile([P, width], f8, tag=tag)
                nc.gpsimd.indirect_dma_start(
                    out=ti[:], out_offset=None, in_=dram[:],
                    in_offset=bass.IndirectOffsetOnAxis(
                        ap=idx_sb[:, c : c + 1], axis=1
                    ),
                    element_offset=elem_off,
                )
                return ti

            # gathers: per tile (in order), per segment: y then interleaved x/w
            g = {}          # (t, seg_idx) -> (y_tile, ilv_tile)
            for t in cfg["tile_order"]:
                for si, (slo, shi) in enumerate(segs[t]):
                    if cfg.get("ilv_first"):
                        it = gather(
                            ilv, 2 * t, 2 * (shi - slo), 2 * slo, f"i{si}"
                        )
                        yt = gather(ydat, 2 * t + 1, shi - slo, slo, f"y{si}")
                    else:
                        yt = gather(ydat, 2 * t + 1, shi - slo, slo, f"y{si}")
                        it = gather(
                            ilv, 2 * t, 2 * (shi - slo), 2 * slo, f"i{si}"
                        )
                    g[(t, si)] = (slo, shi, yt, it)

            def slices(t, lo, hi):
                """(x_ap, w_ap, y_ap) for tile t columns [lo, hi)."""
                for si in range(len(segs[t])):
                    slo, shi, yt, it = g[(t, si)]
                    if lo >= slo and hi <= shi:
                        a, b = lo - slo, hi - slo
                        return (
                            it[:, 2 * a : 2 * b : 2],
                            it[:, 2 * a + 1 : 2 * b : 2],
                            yt[:, a:b],
                        )
                raise AssertionError("chunk straddles a gather segment")

            for gcol, grp in enumerate(groups):
                gw = sum(hi - lo for (_, lo, hi, _) in grp)
                tt = scr.tile([P, gw], f16, tag="tt")
                off = 0
                for (t, lo, hi, tail) in grp:
                    w = hi - lo
                    d = ps.tile([P, w], f32, tag="d")
                    for mlo in range(0, w, cfg["mm_max"]):
                        mhi = min(w, mlo + cfg["mm_max"])
                        x_ap, _, y_ap = slices(t, lo + mlo, lo + mhi)
                        nc.tensor.matmul(
                            d[:, mlo:mhi], id_sb[:, 0:P], x_ap,
                            start=True, stop=False,
                        )
                        nc.tensor.matmul(
                            d[:, mlo:mhi], id_sb[:, P : 2 * P], y_ap,
                            start=False, stop=True,
                        )
                    _, w_ap, _ = slices(t, lo, hi)
                    nc.vector.tensor_tensor(
                        out=tt[:, off : off + w], in0=d[:], in1=w_ap,
                        op=mybir.AluOpType.mult,
                    )
                    off += w
                if grp[0][3]:
                    # drain chunks: square+reduce on DVE, skip the ACT chase
                    sq = scr.tile([P, gw], f16, tag="sq")
                    nc.vector.tensor_tensor(
                        out=sq[:], in0=tt[:], in1=tt[:], op=mybir.AluOpType.mult
                    )
                    nc.vector.tensor_reduce(
                        out=acc[:, gcol : gcol + 1], in_=sq[:],
                        axis=mybir.AxisListType.X, op=mybir.AluOpType.add,
                    )
                else:
                    d2 = scr.tile([P, gw], f16, tag="d2")
                    nc.scalar.activation(
                        out=d2[:], in_=tt[:],
                        func=mybir.ActivationFunctionType.Square,
                        accum_out=acc[:, gcol : gcol + 1],
                    )
                if cfg["split_res"] and gcol == ncol - 2:
                    # bulk result columns leave while the last group drains
                    nc.sync.dma_start(
                        out=res[:, : ncol - 1], in_=acc[:, : ncol - 1]
                    )

            if cfg["split_res"]:
                nc.sync.dma_start(
                    out=res[:, ncol - 1 : ncol], in_=acc[:, ncol - 1 : ncol]
                )
            else:
                nc.sync.dma_start(out=res[:], in_=acc[:])

    legalize_waits(nc)
    return nc, work


def prepare_inputs(fluctuate, ivar, output, overlap_index):
    """Globally sort samples by window length, deal round-robin to cores,
    stage fp8 interleaved x/sqrt(ivar) (window-masked) and fp8 y per core."""
    flu = np.ascontiguousarray(fluctuate.reshape(B, L), dtype=np.float32)
    ivr = np.ascontiguousarray(ivar.reshape(B, L), dtype=np.float32)
    oup = np.ascontiguousarray(output.reshape(B, L), dtype=np.float32)
    oi = np.asarray(overlap_index)
    s_in = oi[:, 0].astype(np.int64)
    e_in = oi[:, 1].astype(np.int64)
    s_out = oi[:, 2].astype(np.int64)
    all_lens = e_in - s_in

    order = np.argsort(-all_lens, kind="stable")   # global, descending
    lens_sorted = all_lens[order]

    # tile t's width: the longest window among ranks [1024t, 1024(t+1))
    widths = []
    for t in range(TILES):
        w = int(lens_sorted[t * P * N_CORES])
        widths.append(min(MAX_W, -(-w // 32) * 32))

    x8 = NP_F8(np.clip(flu, -F8_MAX, F8_MAX))
    y8 = NP_F8(np.clip(oup, -F8_MAX, F8_MAX))
    sw8 = NP_F8(np.sqrt(ivr, dtype=np.float32))

    ident = np.zeros((P, 2 * P), dtype=NP_F8)
    ident[:, :P] = NP_F8(np.eye(P, dtype=np.float32))
    ident[:, P:] = NP_F8(-np.eye(P, dtype=np.float32))

    jj = np.arange(L)
    in_maps = []
    core_lens = []
    for c in range(N_CORES):
        g = order[c::N_CORES]                      # this core's samples, sorted
        core_lens.append(all_lens[g].reshape(TILES, P))

        win = (jj[None, :] >= s_in[g, None]) & (jj[None, :] < e_in[g, None])
        ilv = np.zeros((BPC, ILV_STRIDE), dtype=NP_F8)
        ilv[:, 0 : 2 * L : 2] = x8[g]
        ilv[:, 1 : 2 * L : 2] = np.where(win, sw8[g], NP_F8(0))
        ydat = np.zeros((BPC + 1, L), dtype=NP_F8)
        ydat[:BPC] = y8[g]

        rows = np.arange(BPC)
        idxm = np.empty((P, 2 * TILES), dtype=np.int32)
        for t in range(TILES):
            sl = slice(t * P, (t + 1) * P)
            idxm[:, 2 * t + 0] = rows[sl] * ILV_STRIDE + 2 * s_in[g][sl]
            idxm[:, 2 * t + 1] = rows[sl] * L + s_out[g][sl]

        in_maps.append(
            {"ilv": ilv, "ydat": ydat, "idx": idxm, "ident": ident}
        )

    return in_maps, widths, core_lens


def finish(results, work, core_lens):
    """Combine per-core per-chunk partial sums into the scalar mean."""
    total = 0.0
    for c in range(N_CORES):
        res = results[c]["res"].astype(np.float64)     # [P, ncol]
        sums = np.zeros((TILES, P), dtype=np.float64)
        for (t, lo, hi, col) in work:
            sums[t] += res[:, col]
        lens = core_lens[c].astype(np.float64)
        total += float((sums / lens).sum())
    return np.float32(total / B)


def kernel(fluctuate, ivar, output, overlap_index, _trace=False, **_kw):
    in_maps, widths, core_lens = prepare_inputs(
        fluctuate, ivar, output, overlap_index
    )
    nc, work = build_bass(widths)
    out = run_bass_kernel_spmd(
        nc, in_maps, core_ids=list(range(N_CORES)), trace=_trace
    )
    result = finish(out.results, work, core_lens)
    if _trace:
        return result, out
    return result
